# revision 16
# baseline (speedup 1.0000x reference)
"""Trainium2 Bass kernel for nn_DenoisingSharpening.

Contract: kernel(**inputs) takes the FULL unsharded inputs
(images [8,64,64,64,3] f32, params [8,64,7] f32, k [] f32) and returns
the FULL output [8,64,64,64,3] f32.

Strategy (v2)
-------------
Data-parallel over N = B*P = 512 images; 64 images per NeuronCore, one
half-image (32 rows) per SBUF partition -> 128 partitions.

Host side: reflect-pads each image, converts to f16, de-interleaves the
channels into planes ([128, 3, 34, 68]; 68-wide so f16 rows stay aligned
and even), precomputes per-image scalars, and builds the ±I / diag(bC) /
±diag(bE) stationary matrices for the PE engine. The skip branch is
applied on the host from accumulator sums shipped back by the device.

Device per 8-row chunk (channel-plane tiles, f16 throughout, PSUM f32):
  * 4 neighbor-diff fields (W, NW, N, NE): f16 tensor_tensor (DVE 2x).
  * d2 = sum_c diff_c^2: ACT Square (channel-grouped), two f16 adds;
    ck = Exp(-s^2*d2 + log w) on ACT (scale folds s^2).
  * prod = ck (x) diff via channel-dim broadcast (innermost stays packed).
  * bilateral sums nacc / sacc: PE identity matmuls over shifted views,
    accumulated in PSUM; center weight aC^2 added as ACT bias.
  * gaussian detail: H = dW(c+1)-dW(c) (one f16 TT), then
    inner = bC*H + bE*(H(up)+H(dn)) + dN(dn) - dN via 5 PE diagonal
    matmuls per channel into PSUM.
  * noise/detail masks: ACT chain (Abs/Tanh/Square/Exp, one table set),
    noise mask fused as Square(-sql*ee + sql); skip sums via accum_out.
  * out = clip(x + nacc/sacc + s3*detail) in f16, one f32 cast into a
    resident output tile; one contiguous DMA in, one out per iteration.
"""

import numpy as np

N_CORES = 8
B, PP, H, W, C = 8, 64, 64, 64, 3
NIMG = B * PP  # 512
HALVES = 2 * NIMG  # 1024 half-images, 128 per core
ROWS_PER_HALF = 32
PR = ROWS_PER_HALF + 2  # padded rows (34)
PC = 68  # padded cols: 0 junk, 1 reflect, 2..65 interior, 66 reflect, 67 junk
CHUNKS = 4
CR = ROWS_PER_HALF // CHUNKS  # interior rows per chunk (8)
SLAB_R = CR + 2  # chunk rows incl halo (10)

NOISE_THRESH = 0.002
SKIP_THRESH = 1e-4
MEAN_N = float(C * H * W)

# params columns
(P_S2N, P_LOGE, P_LOGC, P_WSC, P_KT, P_KTB, P_IGT, P_OFFGT, P_CLIP,
 P_SQL, P_NSQL, P_BE, P_PAD1, P_PAD2, P_PAD3, P_PAD4) = range(16)
NPARAM = 16

# stationary weight matrices
(W_IP, W_IN, W_BCP, W_BEP, W_WSC) = range(5)
NWMAT = 5

_CACHE = {}
DEFAULT_CFG = {"sq_dve": 4}


# --------------------------------------------------------------------------
# host-side preprocessing
# --------------------------------------------------------------------------

def _host_prep(images, params, k):
    x = np.ascontiguousarray(images, dtype=np.float32).reshape(NIMG, H, W, C)
    xp = np.pad(x, ((0, 0), (1, 1), (1, 1), (0, 0)), mode="reflect")
    halves = np.stack([xp[:, 0:PR], xp[:, ROWS_PER_HALF:ROWS_PER_HALF + PR]],
                      axis=1).reshape(HALVES, PR, 66, C)
    planes = np.transpose(halves, (0, 3, 1, 2))  # [HALVES, C, PR, 66]
    xpad = np.empty((HALVES, C, PR, PC), np.float16)
    xpad[:, :, :, 1:67] = planes
    xpad[:, :, :, 0] = planes[:, :, :, 0]
    xpad[:, :, :, 67] = planes[:, :, :, 65]

    p = np.asarray(params, dtype=np.float32).reshape(NIMG, 7)
    sigma_s = np.clip(p[:, 0], 0.2, 5.0)
    sigma_r = np.clip(p[:, 1], 0.01, 1.0)
    sigma_f = np.clip(p[:, 2], 0.2, 3.0)
    lam = np.clip(p[:, 3], 0.1, 2.0)
    tau = np.clip(p[:, 4], 0.5, 5.0)
    gain = np.clip(p[:, 5], 0.2, 2.0)
    offset = np.clip(p[:, 6], 0.01, 1.0)

    def gauss1d(sig):
        g = np.exp(-0.5 * (np.array([-1.0, 0.0, 1.0], np.float32)[None, :]
                           / sig[:, None]) ** 2)
        return g / g.sum(axis=1, keepdims=True)

    gs = gauss1d(sigma_s)
    gf = gauss1d(sigma_f)
    aE, aC = gs[:, 0], gs[:, 1]
    bE = gf[:, 0]
    bC = 1.0 - 2.0 * bE

    kpos = max(abs(float(np.asarray(k))), 1.0)
    gt = gain / tau

    pars = np.zeros((NIMG, NPARAM), np.float32)
    pars[:, P_S2N] = -0.5 / sigma_r ** 2
    pars[:, P_LOGE] = np.log(aE * aC)
    pars[:, P_LOGC] = np.log(aE * aE)
    pars[:, P_WSC] = aC * aC
    pars[:, P_KT] = 0.5 * kpos
    pars[:, P_KTB] = -0.5 * kpos * NOISE_THRESH
    pars[:, P_IGT] = 1.0 / gt
    pars[:, P_OFFGT] = offset / gt
    pars[:, P_CLIP] = 10.0 / tau
    pars[:, P_SQL] = np.sqrt(lam / 2.0)
    pars[:, P_NSQL] = -np.sqrt(lam / 2.0)
    pars[:, P_BE] = bE
    pars2 = np.repeat(pars, 2, axis=0)  # [1024, NPARAM]

    bE2 = np.repeat(bE, 2)
    bC2 = np.repeat(bC, 2)

    in_maps = []
    per_core = HALVES // N_CORES
    idx = np.arange(per_core)
    for c in range(N_CORES):
        sl = slice(c * per_core, (c + 1) * per_core)
        wmx = np.zeros((per_core, NWMAT, per_core), np.float16)
        wmx[idx, W_IP, idx] = 1.0
        wmx[idx, W_IN, idx] = -1.0
        wmx[idx, W_BCP, idx] = bC2[sl]
        wmx[idx, W_BEP, idx] = bE2[sl]
        wmx[idx, W_WSC, idx] = np.repeat(pars[:, P_WSC], 2)[sl]
        in_maps.append({
            "xpad": np.ascontiguousarray(xpad[sl]),
            "pp": np.ascontiguousarray(pars2[sl]),
            "wmat": np.ascontiguousarray(wmx),
        })
    return in_maps


def _host_post(results, images=None, params=None):
    outs = [r["out"] for r in results]  # each [128, C, 32, 64] f32
    full = np.concatenate(outs, axis=0)  # [1024, C, 32, 64]
    full = full.reshape(NIMG, 2, C, ROWS_PER_HALF, W)
    full = np.ascontiguousarray(np.transpose(full, (0, 1, 3, 4, 2)))
    full = full.reshape(NIMG, H, W, C)
    if images is not None and all("sksum" in r for r in results):
        sk = np.concatenate([r["sksum"] for r in results], axis=0)  # [1024, 8]
        a_half = sk[:, 0:CHUNKS].sum(axis=1)
        n_half = sk[:, CHUNKS:2 * CHUNKS].sum(axis=1)
        a_img = a_half[0::2] + a_half[1::2]
        n_img = n_half[0::2] + n_half[1::2]
        tau = np.clip(np.asarray(params, np.float32).reshape(NIMG, 7)[:, 4],
                      0.5, 5.0)
        skip = ((a_img < MEAN_N * SKIP_THRESH)
                | (n_img < MEAN_N * SKIP_THRESH / tau))
        if skip.any():
            x = np.asarray(images, np.float32).reshape(NIMG, H, W, C)
            full = full.copy()
            full[skip] = np.clip(x[skip], 1e-5, 1.0)
    return full.reshape(B, PP, H, W, C)


# --------------------------------------------------------------------------
# device program
# --------------------------------------------------------------------------

def build_program(cfg=None):
    import concourse.tile as tile
    from concourse import bacc, mybir
    from contextlib import ExitStack

    cfg = dict(DEFAULT_CFG, **(cfg or {}))
    F32 = mybir.dt.float32
    F16 = mybir.dt.float16
    ALU = mybir.AluOpType
    AF = mybir.ActivationFunctionType
    repeat = int(cfg.get("repeat", 1))
    div_pool = bool(cfg.get("div_pool", True))
    o1_pool = bool(cfg.get("o1_pool", True))
    sq_dve = int(cfg.get("sq_dve", 0))  # number of taps whose square runs on DVE
    d2_pool = int(cfg.get("d2_pool", 0))  # number of taps whose d2-add runs on Pool

    nc = bacc.Bacc("TRN2", target_bir_lowering=False, debug=False)
    xdram = nc.dram_tensor("xpad", [128, C, PR, PC], F16, kind="ExternalInput").ap()
    pdram = nc.dram_tensor("pp", [128, NPARAM], F32, kind="ExternalInput").ap()
    wdram = nc.dram_tensor("wmat", [128, NWMAT, 128], F16,
                           kind="ExternalInput").ap()
    odram = nc.dram_tensor("out", [128, C, ROWS_PER_HALF, W], F32,
                           kind="ExternalOutput").ap()
    skdram = nc.dram_tensor("sksum", [128, 2 * CHUNKS], F32,
                            kind="ExternalOutput").ap()

    # taps: name, dr, dc, tap-grid (slab) row range, col range, log-weight col.
    # Tap value t[q] = x[q + (dr,dc)] - x[q] on grid q; the grid covers both
    # the interior window and its mirror (interior - delta).
    TAPS = [
        ("W", 0, -1, (0, SLAB_R), (1, 67), P_LOGE),   # rows 0..9 for H reuse
        ("NW", -1, -1, (1, SLAB_R), (1, 67), P_LOGC),
        ("N", -1, 0, (1, SLAB_R), (2, 66), P_LOGE),
        ("NE", -1, 1, (1, SLAB_R), (1, 67), P_LOGC),
    ]
    BIL_ROWS = {"W": (1, 9), "NW": (1, SLAB_R), "N": (1, SLAB_R),
                "NE": (1, SLAB_R)}

    with tile.TileContext(nc) as tc:
        with ExitStack() as ctx:
            pool = ctx.enter_context(tc.tile_pool(name="main", bufs=1))
            psum = ctx.enter_context(
                tc.tile_pool(name="psum", bufs=1, space="PSUM"))

            pp = pool.tile([128, NPARAM], F32, tag="pp", bufs=1)
            nc.sync.dma_start(pp[:], pdram[:])
            wmat = pool.tile([128, NWMAT, 128], F16, tag="wmat", bufs=1)
            nc.sync.dma_start(wmat[:], wdram[:])

            def par(col):
                return pp[:, col:col + 1]

            def wm(j):
                return wmat[:, j, :]

            ones = pool.tile([128, CR, W], F16, tag="ones", bufs=1)
            nc.vector.memset(ones[:], 1.0)

            for rep in range(repeat):
                accs = pool.tile([128, 2 * CHUNKS], F32, tag="accs", bufs=2,
                                 name=f"accs{rep}")

                for chk in range(CHUNKS):
                    rbase = chk * CR
                    slab = pool.tile([128, C, SLAB_R, PC], F16, tag="slab",
                                     bufs=2, name=f"slab{chk}_{rep}")
                    nc.sync.dma_start(
                        slab[:], xdram[:, :, rbase:rbase + SLAB_R, :])

                    def xr(r0, r1, c0, c1, slab=slab):
                        return slab[:, :, r0:r1, c0:c1]

                    # ---------------- diff fields ----------------
                    diffs = {}
                    for (name, dr, dc, (rs, re), (cs, ce), bcol) in TAPS:
                        nr, ncol = re - rs, ce - cs
                        d = pool.tile([128, C, nr, ncol], F16,
                                      tag="df" + name, bufs=2,
                                      name=f"df{name}{chk}_{rep}")
                        nc.vector.tensor_tensor(
                            d[:],
                            xr(rs + dr, re + dr, cs + dc, ce + dc),
                            xr(rs, re, cs, ce), ALU.subtract)
                        diffs[name] = d

                    # ---------------- PE: gaussian inner ----------------
                    dW = diffs["W"]
                    dN = diffs["N"]
                    Hf = pool.tile([128, C, SLAB_R, W], F16, tag="Hf", bufs=2,
                                   name=f"Hf{chk}_{rep}")
                    # dW grid origin (0,1): H[r,c] = dW[r,c+1]-dW[r,c]
                    # (pixel cols 2..65 -> dW tile cols 2..65 / 1..64)
                    nc.vector.tensor_tensor(
                        Hf[:], dW[:, :, 0:SLAB_R, 2:2 + W],
                        dW[:, :, 0:SLAB_R, 1:1 + W], ALU.subtract)
                    inner = psum.tile([128, C, CR, W], F32, tag="inner",
                                      bufs=1, name=f"inn{chk}_{rep}")
                    # dN grid origin (1,2): pixel (r, c) -> tile (r-1, c-2)
                    for c in range(C):
                        nc.tensor.matmul(inner[:, c], wm(W_BCP),
                                         Hf[:, c, 1:1 + CR, :],
                                         start=True, stop=False)
                        nc.tensor.matmul(inner[:, c], wm(W_BEP),
                                         Hf[:, c, 0:CR, :],
                                         start=False, stop=False)
                        nc.tensor.matmul(inner[:, c], wm(W_BEP),
                                         Hf[:, c, 2:2 + CR, :],
                                         start=False, stop=False)
                        nc.tensor.matmul(inner[:, c], wm(W_IP),
                                         dN[:, c, 1:1 + CR, 0:W],
                                         start=False, stop=False)
                        nc.tensor.matmul(inner[:, c], wm(W_IN),
                                         dN[:, c, 0:CR, 0:W],
                                         start=False, stop=True)

                    # d1f = x/gt + off/gt (f32), r1 = 1/d1f in place
                    d1f = pool.tile([128, C, CR, W], F32, tag="d1f", bufs=2,
                                    name=f"d1f{chk}_{rep}")
                    nc.scalar.activation(d1f[:], xr(1, 1 + CR, 2, 2 + W),
                                         AF.Identity, bias=par(P_OFFGT),
                                         scale=par(P_IGT))

                    # ---------------- bilateral weights ----------------
                    prods = {}
                    cks = {}
                    for ti, (name, dr, dc, (rs, re), (cs, ce), bcol) in \
                            enumerate(TAPS):
                        nr, ncol = re - rs, ce - cs
                        brs, bre = BIL_ROWS[name]
                        bnr = bre - brs
                        boff = brs - rs
                        d = diffs[name]
                        sq = pool.tile([128, C, SLAB_R - 1, 66], F16, tag="sq",
                                       bufs=2, name=f"sq{name}{chk}_{rep}")
                        sqv = sq[:, :, 0:bnr, 0:ncol]
                        dv = d[:, :, boff:boff + bnr, 0:ncol]
                        if ti < sq_dve:
                            nc.vector.tensor_tensor(sqv, dv, dv, ALU.mult)
                        else:
                            nc.scalar.activation(sqv, dv, AF.Square)
                        d2a = pool.tile([128, SLAB_R - 1, 66], F16, tag="d2a",
                                        bufs=3, name=f"d2a{name}{chk}_{rep}")
                        av = d2a[:, 0:bnr, 0:ncol]
                        nc.vector.tensor_tensor(
                            av, sq[:, 0, 0:bnr, 0:ncol],
                            sq[:, 1, 0:bnr, 0:ncol], ALU.add)
                        d2 = pool.tile([128, SLAB_R - 1, 66], F16, tag="d2",
                                       bufs=3, name=f"d2{name}{chk}_{rep}")
                        bv = d2[:, 0:bnr, 0:ncol]
                        if ti < d2_pool:
                            nc.gpsimd.tensor_tensor(
                                bv, av, sq[:, 2, 0:bnr, 0:ncol], ALU.add)
                        else:
                            nc.vector.tensor_tensor(
                                bv, av, sq[:, 2, 0:bnr, 0:ncol], ALU.add)
                        ck = pool.tile([128, SLAB_R - 1, 66], F16,
                                       tag="ck" + name, bufs=2,
                                       name=f"ck{name}{chk}_{rep}")
                        ckv = ck[:, 0:bnr, 0:ncol]
                        nc.scalar.activation(ckv, bv, AF.Exp, bias=par(bcol),
                                             scale=par(P_S2N))
                        cks[name] = (ck, brs, cs)
                        prod = pool.tile([128, C, bnr, ncol], F16,
                                         tag="pr" + name, bufs=2,
                                         name=f"pr{name}{chk}_{rep}")
                        nc.vector.tensor_tensor(
                            prod[:], dv,
                            ck[:, 0:bnr, 0:ncol].unsqueeze(1).broadcast_to(
                                [128, C, bnr, ncol]), ALU.mult)
                        prods[name] = (prod, brs, cs)

                    # ------------- PE: sacc then nacc accumulation ----------
                    # interior pixel: slab rows 1..8, cols 2..65. For a tap
                    # tile with grid origin (brs, cs): interior index
                    # (1-brs, 2-cs); mirror window = interior - delta.
                    sacc = psum.tile([128, CR, W], F32, tag="sacc", bufs=1,
                                     name=f"sacc{chk}_{rep}")
                    nc.tensor.matmul(sacc[:], wm(W_WSC), ones[:],
                                     start=True, stop=False)
                    mi = 0
                    for sign in (0, 1):
                        for (name, dr, dc, _, _, _) in TAPS:
                            ck, brs, pcs = cks[name]
                            ir, ic = 1 - brs, 2 - pcs
                            r0 = ir - sign * dr
                            c0 = ic - sign * dc
                            nc.tensor.matmul(
                                sacc[:], wm(W_IP),
                                ck[:, r0:r0 + CR, c0:c0 + W],
                                start=False, stop=(mi == 7))
                            mi += 1
                    rS = pool.tile([128, CR, W], F32, tag="rS", bufs=2,
                                   name=f"rS{chk}_{rep}")
                    nc.vector.reciprocal_approx_fast(
                        rS[:].rearrange("p a b -> p (a b)"),
                        sacc[:].rearrange("p a b -> p (a b)"))

                    nacc = psum.tile([128, C, CR, W], F32, tag="nacc", bufs=1,
                                     name=f"nacc{chk}_{rep}")
                    for c in range(C):
                        n_mm = 2 * len(TAPS)
                        mi = 0
                        for sign, wj in ((0, W_IP), (1, W_IN)):
                            for (name, dr, dc, _, _, _) in TAPS:
                                prod, brs, pcs = prods[name]
                                ir, ic = 1 - brs, 2 - pcs
                                r0 = ir - sign * dr
                                c0 = ic - sign * dc
                                nc.tensor.matmul(
                                    nacc[:, c], wm(wj),
                                    prod[:, c, r0:r0 + CR, c0:c0 + W],
                                    start=(mi == 0), stop=(mi == n_mm - 1))
                                mi += 1

                    # ---------------- noise / masks ----------------
                    nc.vector.reciprocal_approx_fast(
                        d1f[:].rearrange("p c a b -> p (c a b)"),
                        d1f[:].rearrange("p c a b -> p (c a b)"))
                    det = pool.tile([128, C, CR, W], F16, tag="det", bufs=2,
                                    name=f"det{chk}_{rep}")
                    nc.scalar.activation(det[:], inner[:], AF.Copy,
                                         scale=par(P_BE))
                    adet = pool.tile([128, C, CR, W], F16, tag="nz", bufs=5,
                                     name=f"adet{chk}_{rep}")
                    nc.scalar.activation(adet[:], inner[:], AF.Abs,
                                         scale=par(P_BE),
                                         accum_out=accs[:, chk:chk + 1])
                    th = pool.tile([128, C, CR, W], F16, tag="th", bufs=2,
                                   name=f"th{chk}_{rep}")
                    nc.scalar.activation(th[:], adet[:], AF.Tanh,
                                         bias=par(P_KTB), scale=par(P_KT))
                    ne0 = pool.tile([128, C, CR, W], F16, tag="nz", bufs=5,
                                    name=f"ne0_{chk}_{rep}")
                    if div_pool:
                        nc.gpsimd.tensor_tensor(ne0[:], adet[:], d1f[:],
                                                ALU.mult)
                    else:
                        nc.vector.tensor_tensor(ne0[:], adet[:], d1f[:],
                                                ALU.mult)
                    neq = pool.tile([128, C, CR, W], F16, tag="nz", bufs=5,
                                    name=f"neq{chk}_{rep}")
                    nc.vector.tensor_scalar(
                        neq[:], ne0[:], par(P_CLIP), None, ALU.min, ALU.add,
                        accum_out=accs[:, CHUNKS + chk:CHUNKS + chk + 1])
                    sqn = pool.tile([128, C, CR, W], F16, tag="nz", bufs=5,
                                    name=f"sqn{chk}_{rep}")
                    if cfg.get("sqn_pool"):
                        nc.gpsimd.tensor_tensor(sqn[:], neq[:], neq[:],
                                                ALU.mult)
                    else:
                        nc.scalar.activation(sqn[:], neq[:], AF.Square)
                    ee = pool.tile([128, C, CR, W], F16, tag="nz", bufs=5,
                                   name=f"ee{chk}_{rep}")
                    nc.scalar.activation(ee[:], sqn[:], AF.Exp, scale=-1.0)
                    nm = pool.tile([128, C, CR, W], F16, tag="nz", bufs=5,
                                   name=f"nm{chk}_{rep}")
                    # nm = (sql*(1-ee))^2 = Square(ee*(-sql) + sql)
                    nc.scalar.activation(nm[:], ee[:], AF.Square,
                                         bias=par(P_SQL), scale=par(P_NSQL))
                    thp = pool.tile([128, C, CR, W], F16, tag="nz", bufs=5,
                                    name=f"thp{chk}_{rep}")
                    nc.vector.tensor_scalar(thp[:], th[:], 1.0, None, ALU.add)
                    s3 = pool.tile([128, C, CR, W], F16, tag="nz", bufs=5,
                                   name=f"s3_{chk}_{rep}")
                    nc.vector.tensor_tensor(s3[:], thp[:], nm[:], ALU.mult)
                    sharp = pool.tile([128, C, CR, W], F16, tag="nz", bufs=5,
                                      name=f"sharp{chk}_{rep}")
                    nc.vector.tensor_tensor(sharp[:], s3[:], det[:], ALU.mult)

                    # ---------------- bf assembly + output ----------------
                    tT = pool.tile([128, C, CR, W], F16, tag="tT", bufs=2,
                                   name=f"tT{chk}_{rep}")
                    nc.vector.tensor_tensor(
                        tT[:], nacc[:],
                        rS[:].unsqueeze(1).broadcast_to([128, C, CR, W]),
                        ALU.mult)
                    o1 = pool.tile([128, C, CR, W], F16, tag="o1", bufs=2,
                                   name=f"o1_{chk}_{rep}")
                    if o1_pool:
                        nc.gpsimd.tensor_tensor(
                            o1[:], tT[:], xr(1, 1 + CR, 2, 2 + W), ALU.add)
                    else:
                        nc.vector.tensor_tensor(
                            o1[:], tT[:], xr(1, 1 + CR, 2, 2 + W), ALU.add)
                    o2 = pool.tile([128, C, CR, W], F16, tag="nz", bufs=5,
                                   name=f"o2_{chk}_{rep}")
                    nc.vector.tensor_tensor(o2[:], o1[:], sharp[:], ALU.add)
                    o3f = pool.tile([128, C, CR, W], F32, tag="o3f", bufs=2,
                                    name=f"o3f{chk}_{rep}")
                    nc.vector.tensor_scalar(o3f[:], o2[:], 1e-5, 1.0,
                                            ALU.max, ALU.min)
                    nc.sync.dma_start(odram[:, :, rbase:rbase + CR, :], o3f[:])

                nc.sync.dma_start(skdram[:], accs[:])

    nc.compile()
    return nc


def _get_program(cfg=None):
    key = tuple(sorted((cfg or {}).items()))
    if key not in _CACHE:
        _CACHE[key] = build_program(cfg)
    return _CACHE[key]


# --------------------------------------------------------------------------
# entry point
# --------------------------------------------------------------------------

def kernel(images, params, k):
    from concourse.bass_utils import run_bass_kernel_spmd

    nc = _get_program({})
    in_maps = _host_prep(np.asarray(images), np.asarray(params), np.asarray(k))
    res = run_bass_kernel_spmd(nc, in_maps, list(range(N_CORES)))
    return _host_post(res.results, images, params).astype(np.float32)
